# revision 1
# baseline (speedup 1.0000x reference)
"""Trainium2 Bass kernel for the DSS (Diagonal State Space) layer.

y = irfft(rfft(u, 2L) * rfft(K, 2L))[:L] + D*u, with K the length-L DSS kernel
derived from (Lambda, W, log_step) via a complex softmax.

Implementation: the FFT convolution is mathematically a causal conv with an
exponentially-structured kernel K[s] = Re(sum_n wt_n * r_n^s).  We evaluate it
as a chunked diagonal-SSM scan on-device:
  - time-major layout, chunks of C=256 timesteps (2 partition tiles of 128)
  - intra-chunk contribution: Toeplitz-block matmuls (TD diag block, TU upper)
  - inter-chunk contribution: rank-128 state S (Re/Im of 64 complex modes),
    updated per chunk as S' = MT.T S + AA.T u_chunk, applied as VV.T S
  - D*u folded onto the Toeplitz diagonal
All matmuls in float32r (fp32 with 12-bit-truncated mantissa): full PE speed,
and the HW matmul is exact for pre-rounded inputs (error == input rounding,
~1.2e-4 relative).

Sharding: data-parallel over batch; each of 8 cores gets 512 sequences
(time-major 4096x512 shard).  SSM params are tiny and replicated as seven
128x128 constant matrices computed on host in float64.
"""

import os
import sys

for _p in ("/opt/trn_rl_repo",):
    if _p not in sys.path and os.path.isdir(_p):
        sys.path.append(_p)

import numpy as np

EPS = 1e-7          # complex_softmax eps
B, L, N = 4096, 4096, 64
N_CORES = 8
BC = B // N_CORES   # 512 sequences per core
C = 256             # timesteps per chunk
NB = L // C         # 16 chunks
P = 128             # partitions

_CNAMES = ("TD", "TU", "AA0", "AA1", "MT", "VV0", "VV1")

_PROG = None        # compiled Bass program, built once per process


def _round_fp32r(x: np.ndarray) -> np.ndarray:
    """Round fp32 array to fp32r: keep 11 mantissa bits (round-half-even)."""
    b = np.ascontiguousarray(x, dtype=np.float32).view(np.uint32)
    low = b & np.uint32(0xFFF)
    hi = b & np.uint32(0xFFFFF000)
    half = np.uint32(0x800)
    rnd = (low > half) | ((low == half) & (((hi >> np.uint32(12)) & np.uint32(1)) == 1))
    out = hi + np.where(rnd, np.uint32(0x1000), np.uint32(0))
    return out.view(np.float32)


def _constants(Lambda_re, Lambda_im, W, D, log_step):
    """Seven 128x128 fp32r matrices, computed in float64 on host."""
    step = float(np.exp(np.float64(log_step[0])))
    Lam = Lambda_re.astype(np.float64) + 1j * Lambda_im.astype(np.float64)   # (N,)
    Wc = W[0, :, 0].astype(np.float64) + 1j * W[0, :, 1].astype(np.float64)  # (N,)
    s = np.arange(C + 1, dtype=np.float64)
    pows = np.exp(np.outer(s, step * Lam))                                   # (C+1, N)
    Gamma = pows[C]
    sl = np.arange(L, dtype=np.float64)
    Sigma = np.exp(np.outer(sl, step * Lam)).sum(axis=0)                     # (N,)
    wt = (Wc / Lam) * np.conj(Sigma) / (Sigma * np.conj(Sigma) + EPS)
    K = (pows[:C] * wt[None, :]).sum(axis=1).real                            # (C,)

    idx = np.arange(P)
    qp = idx[None, :] - idx[:, None]                                         # q - p
    TD = np.where(qp >= 0, K[np.clip(qp, 0, C - 1)], 0.0)
    TD = TD + np.eye(P) * np.float64(D[0])
    TU = K[qp + P]                                                           # q-p+128 in [1,255]
    AP_ = pows[C - 1 - np.arange(C)]                                         # (C, N) r^{C-1-p}
    AA = np.concatenate([AP_.real, AP_.imag], axis=1)                        # (C, 128)
    MT = np.zeros((P, P), dtype=np.float64)
    n = np.arange(N)
    MT[n, n] = Gamma.real
    MT[64 + n, n] = -Gamma.imag
    MT[n, 64 + n] = Gamma.imag
    MT[64 + n, 64 + n] = Gamma.real
    Vq = pows[1:C + 1] * wt[None, :]                                         # (C, N) wt*r^{q+1}
    VV = np.concatenate([Vq.real.T, -Vq.imag.T], axis=0)                     # (128, C)
    mats = {
        "TD": TD, "TU": TU,
        "AA0": AA[:P], "AA1": AA[P:],
        "MT": MT,
        "VV0": VV[:, :P], "VV1": VV[:, P:],
    }
    return {k: _round_fp32r(v.astype(np.float32)) for k, v in mats.items()}


def _build():
    import concourse.tile as tile
    from concourse import bacc, mybir
    from contextlib import ExitStack

    f32r, f32 = mybir.dt.float32r, mybir.dt.float32
    nc = bacc.Bacc("TRN2", target_bir_lowering=False, debug=False,
                   num_devices=N_CORES)
    ut = nc.dram_tensor("ut", [L, BC], f32r, kind="ExternalInput").ap()
    yt = nc.dram_tensor("yt", [L, BC], f32, kind="ExternalOutput").ap()
    cap = {name: nc.dram_tensor(name, [P, P], f32r, kind="ExternalInput").ap()
           for name in _CNAMES}

    with tile.TileContext(nc) as tc, ExitStack() as ctx:
        cpool = ctx.enter_context(tc.tile_pool(name="const", bufs=1))
        upool = ctx.enter_context(tc.tile_pool(name="u", bufs=8))
        spool = ctx.enter_context(tc.tile_pool(name="s", bufs=3))
        ypool = ctx.enter_context(tc.tile_pool(name="y", bufs=6))
        pypool = ctx.enter_context(tc.tile_pool(name="psy", bufs=4, space="PSUM"))
        pspool = ctx.enter_context(tc.tile_pool(name="pss", bufs=2, space="PSUM"))

        ct = {}
        for name in _CNAMES:
            t = cpool.tile([P, P], f32r, tag=name)
            nc.sync.dma_start(t[:], cap[name])
            ct[name] = t

        s_prev = None
        for J in range(NB):
            first, last = (J == 0), (J == NB - 1)
            u0 = upool.tile([P, BC], f32r, tag="u")
            nc.sync.dma_start(u0[:], ut[J * C: J * C + P, :])
            u1 = upool.tile([P, BC], f32r, tag="u")
            nc.sync.dma_start(u1[:], ut[J * C + P: (J + 1) * C, :])

            psY0 = pypool.tile([P, BC], f32, tag="psy")
            psY1 = pypool.tile([P, BC], f32, tag="psy")
            nc.tensor.matmul(psY0[:], ct["TD"][:], u0[:], start=True, stop=first)
            nc.tensor.matmul(psY1[:], ct["TU"][:], u0[:], start=True, stop=False)
            nc.tensor.matmul(psY1[:], ct["TD"][:], u1[:], start=False, stop=first)
            if not last:
                psS = pspool.tile([P, BC], f32, tag="pss")
                nc.tensor.matmul(psS[:], ct["AA0"][:], u0[:], start=True, stop=False)
                nc.tensor.matmul(psS[:], ct["AA1"][:], u1[:], start=False, stop=first)
            if not first:
                nc.tensor.matmul(psY0[:], ct["VV0"][:], s_prev[:], start=False, stop=True)
                nc.tensor.matmul(psY1[:], ct["VV1"][:], s_prev[:], start=False, stop=True)
                if not last:
                    nc.tensor.matmul(psS[:], ct["MT"][:], s_prev[:], start=False, stop=True)
            if not last:
                s_new = spool.tile([P, BC], f32r, tag="s")
                nc.vector.tensor_copy(s_new[:], psS[:])
                s_prev = s_new

            y0 = ypool.tile([P, BC], f32, tag="y")
            nc.scalar.copy(y0[:], psY0[:])
            y1 = ypool.tile([P, BC], f32, tag="y")
            nc.vector.tensor_copy(y1[:], psY1[:])
            nc.sync.dma_start(yt[J * C: J * C + P, :], y0[:])
            nc.sync.dma_start(yt[J * C + P: (J + 1) * C, :], y1[:])

    nc.compile()
    return nc


def _program():
    global _PROG
    if _PROG is None:
        _PROG = _build()
    return _PROG


# Set PROFILE=True before calling kernel() to capture an NTFF profile;
# LAST_EXEC_NS then holds the measured hardware execution time.
PROFILE = False
LAST_EXEC_NS = None


def kernel(u, Lambda_re, Lambda_im, W, D, log_step):
    global LAST_EXEC_NS
    from concourse.bass_utils import run_bass_kernel_spmd

    u = np.asarray(u, dtype=np.float32)
    consts = _constants(np.asarray(Lambda_re), np.asarray(Lambda_im),
                        np.asarray(W), np.asarray(D), np.asarray(log_step))
    nc = _program()

    in_maps = []
    for c in range(N_CORES):
        utc = _round_fp32r(np.ascontiguousarray(u[c * BC:(c + 1) * BC, :].T))
        in_maps.append({"ut": utc, **consts})

    res = run_bass_kernel_spmd(nc, in_maps, list(range(N_CORES)), trace=PROFILE)
    if PROFILE:
        LAST_EXEC_NS = res.exec_time_ns

    y = np.empty((B, L), dtype=np.float32)
    for c in range(N_CORES):
        y[c * BC:(c + 1) * BC, :] = res.results[c]["yt"].T
    return y
